# revision 36
# baseline (speedup 1.0000x reference)
"""ButterFlyNet2D forward on 8 trn2 NeuronCores.

Sharding: core c handles layer-1 parent block (u0,v0) = divmod(c//2, 2) and
m-half ly = c%2 (2 of 16 layer-2 subtrees), full batch. Butterfly weights are
read exactly once across the 8 cores.

Numerics: single-pass bf16 matmuls with fp32 PSUM accumulation for all
layers (weights and activations rounded to bf16). Measured rel err vs the
fp32 reference ~7e-3 (gate 2e-2).

Activation layout: SBUF tiles [128 partitions = 64*q + n, cols], where (p, q)
are the 2x2 patch offsets of the NEXT conv layer (p picks the tile set, q the
partition half); cols = block-major: ib*npos + b*(s/2)^2 + y2*(s/2) + x2,
chopped into [128, 2048] tiles. The first layer is block-diagonal over X
parity so its psum scatter is partition-preserving (full 128-part copies).

Weights stream from DRAM as one concatenated [128, 87296] bf16 tensor: a
resident l1+l2 slab plus 21 x 1 MB chunks in a deep rotation, spread over
three DMA queues (SP/ACT HWDGE + SWDGE) so issue gaps overlap. Fills run on
a LAG-1 diagonal schedule (wave F = l2fF, l3f(F-1), l4f(F-2), l5f(F-3)) with
the chunk layout in consumption order, overlapping l4/l5 compute with the
weight-stream tail while keeping a full wave of slack on every producer-
consumer edge. Psum->act scatters alternate between ACT and DVE.
"""

import numpy as np
import ml_dtypes

# ---------------------------------------------------------------- constants
B, C, H, W, L, T = 32, 1, 64, 64, 6, 4
NCH = 64
KO = 256
N_CORES = 8
FILL_W = 2048
TILE_W = 2048

LAYER_S = {1: 32, 2: 16, 3: 8, 4: 4, 5: 2}
LAYER_NPOS = {l: 32 * (LAYER_S[l] // 2) ** 2 for l in LAYER_S}
LAYER_M = {1: 128, 2: 256, 3: 256, 4: 256, 5: 256}
LAYER_NB = {1: 1, 2: 2, 3: 8, 4: 32, 5: 128}
NTILES = {1: 4, 2: 2, 3: 2, 4: 2, 5: 2}  # act tiles per p

CHUNK_W = 6144
CHUNK0_W = 1280            # l1+l2 slab, loaded once outside the rotation
N_CHUNKS = 14              # uniform chunks after the slab
WC_BUFS = 7
NSPLIT = 1  # psum fills per 2048-col group (1 = full [128,2048] fills)
WCAT_W = CHUNK0_W + N_CHUNKS * CHUNK_W
SF_W = 21504               # l3|l4|l5 weight cols consumed per super-fill


# diagonal (LAG-1) fill schedule: wave F runs l2fF, l3f(F-1), l4f(F-2),
# l5f(F-3); weights are laid out in that consumption order
DIAG = [(3, 0), (3, 1), (4, 0), (3, 2), (4, 1), (5, 0),
        (3, 3), (4, 2), (5, 1), (4, 3), (5, 2), (5, 3)]
PF = {3: 2, 4: 8, 5: 32}        # weight blocks per fill
GB = {}
_off = 0
for (_l, _f) in DIAG:
    GB[(_l, _f)] = _off
    _off += PF[_l] * 512


def wbase(l, ib):
    """wcat col base of block ib's [128, 2M] slab (diagonal order)."""
    if l == 1:
        return 0
    if l == 2:
        return 256 + ib * 512
    pf = PF[l]
    return CHUNK0_W + GB[(l, ib // pf)] + (ib % pf) * 512

BF16 = ml_dtypes.bfloat16


def core_geom(c):
    P, ly = divmod(c, 2)
    u0, v0 = divmod(P, 2)
    blocks = {1: [(u0, v0)]}
    for l in range(2, 6):
        ms = [ly] if l - 1 == 1 else [0, 1]
        nxt = []
        for (u, v) in blocks[l - 1]:
            for a in ms:
                for bb in (0, 1):
                    nxt.append((2 * u + a, 2 * v + bb))
        blocks[l] = nxt
    return u0, v0, ly, blocks


def m_list(l, ly):
    return [ly] if l == 1 else [0, 1]


def child_index(l, ib, m, klx):
    return klx if l == 1 else ib * 4 + m * 2 + klx


def prod_units(l, ly):
    return [(ib, m) for ib in range(LAYER_NB[l]) for m in m_list(l, ly)]


# ---------------------------------------------------------------- host packing
def pack_weights_layer(Wl, l, blocks_l, ly):
    """-> [nb, 128, 2*M] bf16; partition = 64*q + n; free = p*M + k."""
    M = LAYER_M[l]
    out = np.zeros((len(blocks_l), 128, 2 * M), dtype=BF16)
    for i, (u, v) in enumerate(blocks_l):
        wb = np.asarray(Wl[0, u, v], dtype=np.float32)      # [256, 64, 2, 2]
        if l == 1:
            wb = wb[ly * 128:(ly + 1) * 128]
        wt = wb.transpose(3, 1, 2, 0).reshape(128, 2, M)     # (q,n), p, k
        out[i] = wt.reshape(128, 2 * M).astype(BF16)
    return out


def pack_wcat(inputs, blocks, ly):
    """-> [128, WCAT_W] bf16: l1+l2 slab, then per-super-fill l3|l4|l5."""
    out = np.zeros((128, WCAT_W), dtype=BF16)
    for l in range(1, 6):
        wl = pack_weights_layer(inputs[f"W{l}"], l, blocks[l], ly)
        nb, _, w2m = wl.shape
        for ib in range(nb):
            b = wbase(l, ib)
            out[:, b: b + w2m] = wl[ib]
    return out


def pack_first(W0, u0, v0):
    """-> lhsT [8, 128] bf16, block-diag over the 2 Y-half chunks."""
    koff = (u0 * 2 + v0) * 64
    w0e = np.asarray(W0[0, koff:koff + 64, 0], dtype=np.float32)  # [64, 2, 2]
    wt = w0e.reshape(64, 4).T.astype(BF16)                        # [4(hw), 64]
    out = np.zeros((8, 128), dtype=BF16)
    out[0:4, 0:64] = wt
    out[4:8, 64:128] = wt
    return out


def pack_patches(x):
    """-> [8, 16384] bf16; chunk i = batch i, col within chunk = Y*16 + X2;
    K rows 0:4 = X-even patches, 4:8 = X-odd (block-diag by X parity, so the
    first-layer psum partition halves line up with the act q-halves)."""
    xs = np.asarray(x[:, 0], dtype=np.float32)
    p = xs.reshape(B, 32, 2, 32, 2).transpose(2, 4, 0, 1, 3).reshape(4, B, 32, 32)
    p = p.astype(BF16)
    out = np.zeros((8, 16384), dtype=BF16)
    for i in range(32):
        for cp in range(2):
            sl = p[:, i, :, cp::2]                     # [4, 32 Y, 16 X2]
            out[cp * 4:(cp + 1) * 4, i * 512:(i + 1) * 512] = sl.reshape(4, 512)
    return out


def pack_wf(Wf, blocks5):
    """-> [128, 2048] bf16 block-diag pairs; slot idx: cols [8*idx, 8*idx+8),
    rows 0:64 = Wf(klx=0 block).T at cols 0:4, rows 64:128 = klx=1 at 4:8."""
    out = np.zeros((128, 2048), dtype=np.float32)
    for idx in range(256):
        ib, m = idx // 2, idx % 2
        u, v = blocks5[ib]
        for klx in range(2):
            wft = np.asarray(Wf[0, 2 * u + m, 2 * v + klx], np.float32)  # [4,64]
            out[klx * 64:(klx + 1) * 64,
                idx * 8 + klx * 4:idx * 8 + klx * 4 + 4] = wft.T
    return out.astype(BF16)


# ------------------------------------------------------- scatter descriptors
# copy = (src_pbase, src_off, src_ap, dst_pbase, dst_off_rel, dst_ap[, pcount])
# region = dict(p2, g, dst_start(local col in tile), width, copies)
def first_fill_descs(f):
    """psum cols = (b 4, Y 32, X2 16); partitions = (Xpar, k). One
    partition-preserving 128-part copy per (fill, p2=Y parity)."""
    regions = []
    for p2 in range(2):
        base = 1024 * f
        copies = [(0, p2 * 16, [[512, 4], [32, 16], [1, 16]],
                   0, 0, [[256, 4], [16, 16], [1, 16]], 128)]
        regions.append(dict(p2=p2, g=base // TILE_W, dst_start=base % TILE_W,
                            width=1024, copies=copies))
    return regions


def layer_fill_descs(l, f, ly):
    s2 = LAYER_S[l] // 2
    npos_next = (32 * s2 * s2) // 4
    units = prod_units(l, ly)
    regions = []
    for p2 in range(2):
        reg_map = {}

        def add(g, dst_global, copy):
            reg = reg_map.setdefault(g, dict(p2=p2, g=g, copies=[], _glob=[]))
            reg["copies"].append(copy)
            reg["_glob"].append(dst_global)

        for klx in range(2):
            for q2 in range(2):
                if l == 1:
                    ib, m = units[0]
                    ibc = child_index(l, ib, m, klx)
                    dg = ibc * npos_next + 8 * f * 64
                    add(dg // TILE_W, dg,
                        (klx * 64, p2 * 16 + q2, [[256, 8], [32, 8], [2, 8]],
                         q2 * 64, dg, [[64, 8], [8, 8], [1, 8]]))
                elif l == 2:
                    ib, m = units[f]
                    ibc = child_index(l, ib, m, klx)
                    dg = ibc * npos_next
                    add(dg // TILE_W, dg,
                        (klx * 64, p2 * 8 + q2, [[64, 32], [16, 4], [2, 4]],
                         q2 * 64, dg, [[16, 32], [4, 4], [1, 4]]))
                elif l == 3:
                    ib0, m0 = units[4 * f]
                    ibc0 = child_index(l, ib0, m0, klx)
                    # y2 folded into the middle dim: src col = 512a + 8m + 2c
                    # with m = 2b + y2 (b the 16-stride dim), dst col = 256a
                    # + 2m + c — both uniform, so one copy covers both y2
                    dg = ibc0 * npos_next
                    add(dg // TILE_W, dg,
                        (klx * 64, p2 * 4 + q2,
                         [[512, 4], [8, 64], [2, 2]],
                         q2 * 64, dg, [[256, 4], [2, 64], [1, 2]]))
                elif l == 4:
                    ib0, m0 = units[16 * f]
                    ibc0 = child_index(l, ib0, m0, klx)
                    dg = ibc0 * npos_next
                    add(dg // TILE_W, dg,
                        (klx * 64, p2 * 2 + q2, [[128, 16], [4, 32]],
                         q2 * 64, dg, [[64, 16], [1, 32]]))
                else:
                    raise AssertionError(l)
        for reg in reg_map.values():
            base = min(reg["_glob"])
            ext = 0
            fixed = []
            for (spb, soff, sap, dpb, dg, dap), g0 in zip(reg["copies"],
                                                          reg["_glob"]):
                rel = g0 - base
                fixed.append((spb, soff, sap, dpb, rel, dap))
                ext = max(ext, rel + sum(st * (ct - 1) for st, ct in dap) + 1)
            assert (base % TILE_W) + ext <= TILE_W, (l, f, base, ext)
            regions.append(dict(p2=reg["p2"], g=reg["g"],
                                dst_start=base % TILE_W, width=ext,
                                copies=fixed))
        del reg_map
    return regions


def layer_slots(l, ly):
    npos = LAYER_NPOS[l]
    nch = max(1, npos // 512)
    return [(ib, m, chk) for (ib, m) in prod_units(l, ly) for chk in range(nch)]


def wchunk_of(l, ib):
    """-> (chunk index, col base within chunk) of block ib's [128, 2M] slab.
    Chunk 0 is the resident l1+l2 slab; chunks 1.. rotate through the pool."""
    base = wbase(l, ib)
    if base < CHUNK0_W:
        return 0, base
    return 1 + (base - CHUNK0_W) // CHUNK_W, (base - CHUNK0_W) % CHUNK_W


# ------------------------------------------------------------------ mirror
def _ap_cols(off, ap):
    idx = np.zeros((1,), np.int64) + off
    for stride, count in ap:
        idx = (idx[:, None] + (np.arange(count) * stride)[None, :]).reshape(-1)
    return idx


def mirror_core(inputs, c):
    """Pure-numpy mirror of the device plan for core c -> fout [4,8,2048]."""
    u0, v0, ly, blocks = core_geom(c)
    w0 = pack_first(inputs["W0"], u0, v0).astype(np.float32)
    pat = pack_patches(inputs["input_data"]).astype(np.float32)
    wcat = pack_wcat(inputs, blocks, ly).astype(np.float32)
    wf = pack_wf(inputs["Wf"], blocks[5]).astype(np.float32)

    act = {l: [[np.zeros((128, TILE_W), np.float32) for _ in range(NTILES[l])]
               for _ in range(2)] for l in range(1, 6)}
    fact = [None] * 4

    def apply_regions(psum, regions, l_next):
        for reg in regions:
            for cp in reg["copies"]:
                (spb, soff, sap, dpb, doff, dap) = cp[:6]
                pc = cp[6] if len(cp) > 6 else 64
                sc = _ap_cols(soff, sap)
                dc = _ap_cols(reg["dst_start"] + doff, dap)
                vals = np.maximum(psum[spb:spb + pc][:, sc], 0.0)
                vals = vals.astype(BF16).astype(np.float32)
                act[l_next][reg["p2"]][reg["g"]][dpb:dpb + pc][:, dc] = vals

    for f in range(8):
        psum = np.zeros((128, FILL_W), np.float32)
        for s in range(4):
            t = 4 * f + s
            psum[:, s * 512:(s + 1) * 512] = w0.T @ pat[:, t * 512:(t + 1) * 512]
        apply_regions(psum, first_fill_descs(f), 1)

    for l in range(1, 6):
        M = LAYER_M[l]
        npos = LAYER_NPOS[l]
        slots = layer_slots(l, ly)
        w_slot = min(npos, 512)
        spf = FILL_W // w_slot
        nfill = len(slots) // spf
        for f in range(nfill):
            psum = np.zeros((128, FILL_W), np.float32)
            for si in range(spf):
                ib, m, chk = slots[f * spf + si]
                colg = ib * npos + chk * 512
                g, loc = colg // TILE_W, colg % TILE_W
                base = wbase(l, ib)
                wb = wcat[:, base: base + 2 * M]
                mh = m * 128 if M == 256 else 0
                out = np.zeros((128, w_slot), np.float32)
                for p in range(2):
                    Wh = wb[:, p * M + mh:p * M + mh + 128]
                    Ah = act[l][p][g][:, loc:loc + w_slot]
                    out += Wh.T @ Ah
                psum[:, si * w_slot:(si + 1) * w_slot] = out
            if l == 5:
                fact[f] = np.maximum(psum, 0.0).astype(BF16).astype(np.float32)
            else:
                apply_regions(psum, layer_fill_descs(l, f, ly), l + 1)

    fout = np.zeros((4, 8, FILL_W), np.float32)
    for fi in range(4):
        for s in range(64):
            idx = 64 * fi + s
            rhs = fact[fi][:, s * 32:(s + 1) * 32]
            lhsT = wf[:, idx * 8:idx * 8 + 8]
            fout[fi, :, s * 32:(s + 1) * 32] = np.maximum(lhsT.T @ rhs, 0.0)
    return fout


def decode_outputs(fouts):
    out = np.zeros((B, C, 2, 64, 64), np.float32)
    for c, fo in fouts.items():
        _, _, _, blocks = core_geom(c)
        blocks5 = blocks[5]
        for fi in range(4):
            for s in range(64):
                idx = 64 * fi + s
                ib, m = idx // 2, idx % 2
                u, v = blocks5[ib]
                for klx in range(2):
                    U, V = 2 * u + m, 2 * v + klx
                    yf = fo[fi, klx * 4:klx * 4 + 4, s * 32:(s + 1) * 32]
                    out[:, 0, 0, U, V] = yf[0] - yf[2]
                    out[:, 0, 1, U, V] = yf[1] - yf[3]
    return out


def mirror_forward(inputs, cores=range(N_CORES)):
    return decode_outputs({c: mirror_core(inputs, c) for c in cores})


# ------------------------------------------------------------- numpy fallback
def _numpy_reference(inputs):
    x = np.asarray(inputs["input_data"], np.float32)
    b, c_, h, w = x.shape
    xs = np.zeros((b, c_, 4, h, w), np.float32)
    xs[:, :, 0] = x
    p = xs.reshape(b, c_, 4, 32, 2, 32, 2)
    W0 = np.asarray(inputs["W0"], np.float32)
    b0 = np.asarray(inputs["b0"], np.float32)
    y = np.einsum('bcnYhXw,cknhw->bckYX', p, W0) + b0[None, :, :, None, None]
    state = np.maximum(y, 0).reshape(b, c_, 2, 2, NCH, 32, 32)
    for l in range(1, 6):
        Wl = np.asarray(inputs[f"W{l}"], np.float32)
        bl = np.asarray(inputs[f"b{l}"], np.float32)
        G = Wl.shape[1]
        s = state.shape[-1]
        s2 = s // 2
        p = state.reshape(b, c_, G, G, NCH, s2, 2, s2, 2)
        y = np.einsum('bcuvnYpXq,cuvknpq->bcuvkYX', p, Wl) + \
            bl[None, :, :, :, :, None, None]
        y = np.maximum(y, 0).reshape(b, c_, G, G, 2, 2, NCH, s2, s2)
        y = y.transpose(0, 1, 2, 4, 3, 5, 6, 7, 8)
        state = y.reshape(b, c_, 2 * G, 2 * G, NCH, s2, s2)
    st = state.reshape(b, c_, 64, 64, NCH)
    Wf = np.asarray(inputs["Wf"], np.float32)
    bf = np.asarray(inputs["bf"], np.float32)
    yf = np.maximum(np.einsum('bcuvn,cuvkn->bcuvk', st, Wf) + bf[None], 0)
    real = yf[..., 0] - yf[..., 2]
    imag = yf[..., 1] - yf[..., 3]
    return np.stack([real, imag], axis=2)


# ------------------------------------------------------------- bass program
_NC_CACHE = {}


def build_nc(stop_after=None, loop=False):
    import concourse.bass as bass
    import concourse.mybir as mybir
    import concourse.tile as tile
    from concourse import bacc
    import contextlib

    F32 = mybir.dt.float32
    BF = mybir.dt.bfloat16
    Relu = mybir.ActivationFunctionType.Relu

    nc = bacc.Bacc(None, target_bir_lowering=False, debug=True)

    d_pat = nc.dram_tensor("patches", [8, 16384], BF, kind="ExternalInput")
    d_w0 = nc.dram_tensor("w0", [8, 128], BF, kind="ExternalInput")
    d_wcat = nc.dram_tensor("wcat", [128, WCAT_W], BF,
                            kind="ExternalInput")
    d_wf = nc.dram_tensor("wf", [128, 2048], BF, kind="ExternalInput")
    d_out = nc.dram_tensor("fout", [4, 8, FILL_W], F32, kind="ExternalOutput")
    if loop:
        d_bound = nc.dram_tensor("bound", [1, 1], mybir.dt.int32,
                                 kind="ExternalInput")

    with tile.TileContext(nc) as tc:
        with contextlib.ExitStack() as ctx:
            ps = ctx.enter_context(tc.tile_pool(name="ps", bufs=2, space="PSUM"))
            sb = ctx.enter_context(tc.tile_pool(name="sb", bufs=1))
            wpool = ctx.enter_context(tc.tile_pool(name="wp", bufs=1))

            loop_cm = contextlib.nullcontext()
            if loop:
                bt = sb.tile([1, 1], mybir.dt.int32, tag="bt", bufs=1)
                nc.sync.dma_start(out=bt[:], in_=d_bound[:])
                nval = nc.values_load(bt[0:1, 0:1], min_val=0, max_val=1000000,
                                      skip_runtime_bounds_check=True)
                loop_cm = tc.For_i(0, nval, 1)
            ctx.enter_context(loop_cm)

            w0_sb = sb.tile([8, 128], BF, tag="w0", bufs=1)
            nc.gpsimd.dma_start(out=w0_sb[:], in_=d_w0[:])
            pat_sb = []
            for i in range(4):
                t = sb.tile([8, 4096], BF, tag="mid", bufs=5, name=f"pat{i}")
                nc.gpsimd.dma_start(out=t[:],
                                    in_=d_pat[:, i * 4096:(i + 1) * 4096])
                pat_sb.append(t)
            wf_sb = sb.tile([128, 2048], BF, tag="wf", bufs=1)
            nc.gpsimd.dma_start(out=wf_sb[:], in_=d_wf[:])
            wc0 = wpool.tile([128, CHUNK0_W], BF, tag="wc0", bufs=1,
                             name="wc0")
            nc.sync.dma_start(out=wc0[:], in_=d_wcat[:, 0:CHUNK0_W])
            wc_tiles = [wc0]
            for ci in range(N_CHUNKS):
                t = wpool.tile([128, CHUNK_W], BF, tag="wc", bufs=WC_BUFS,
                               name=f"wc{ci + 1}")
                lo = CHUNK0_W + ci * CHUNK_W
                eng = [nc.sync, nc.scalar, nc.gpsimd][ci % 3]
                eng.dma_start(out=t[:], in_=d_wcat[:, lo:lo + CHUNK_W])
                wc_tiles.append(t)

            act = {l: [[None] * NTILES[l] for _ in range(2)]
                   for l in range(1, 6)}

            def act_tile(l, p, g):
                if act[l][p][g] is None:
                    act[l][p][g] = sb.tile(
                        [128, TILE_W], BF, tag="act", bufs=14,
                        name=f"act{l}_{p}{g}")
                return act[l][p][g]

            sc_count = [0]
            SUBW = FILL_W // NSPLIT
            PS_BUFS = 2 * NSPLIT

            def relu_copy(dst, src):
                # GPSIMD cannot read PSUM, so only ACT and DVE rotate here
                if sc_count[0] % 2 == 0:
                    nc.scalar.activation(dst, src, Relu)
                else:
                    nc.vector.tensor_scalar_max(dst, src, 0.0)
                sc_count[0] += 1

            def emit_scatter_sub(psum_h, h, regions, l_next):
                """Scatter sub-fill h from its [128, SUBW] psum tile. Copy
                APs split on the outermost dim (whose stride*count ==
                FILL_W); src offsets are identical on each sub tile since
                s0*c0/NSPLIT == SUBW."""
                for reg in regions:
                    p2, g, st_loc = reg["p2"], reg["g"], reg["dst_start"]
                    ah = act_tile(l_next, p2, g)
                    for cp in reg["copies"]:
                        (spb, soff, sap, dpb, doff, dap) = cp[:6]
                        pc = cp[6] if len(cp) > 6 else 64
                        (s0, c0), srest = sap[0], sap[1:]
                        (d0, dc0), drest = dap[0], dap[1:]
                        assert s0 * c0 == FILL_W and dc0 == c0, (sap, dap)
                        src = bass.AP(
                            tensor=psum_h[:].tensor,
                            offset=psum_h[:].offset + spb * SUBW + soff,
                            ap=[[SUBW, pc], [s0, c0 // NSPLIT]] +
                               [list(x) for x in srest])
                        dst = bass.AP(
                            tensor=ah[:].tensor,
                            offset=(ah[:].offset + dpb * TILE_W + st_loc +
                                    doff + h * d0 * (c0 // NSPLIT)),
                            ap=[[TILE_W, pc], [d0, c0 // NSPLIT]] +
                               [list(x) for x in drest])
                        relu_copy(dst, src)

            def emit_stop(l_next):
                """Truncated build for phase timing: flush 4 rows of the
                just-written act tiles to d_out so outputs exist."""
                for i in range(4):
                    src = act_tile(l_next, i % 2, 0)
                    nc.gpsimd.dma_start(out=d_out[i], in_=src[0:8, :])

            if stop_after == "dma":
                # DMA-only floor: consume nothing, just flush a weight tile
                for i in range(4):
                    nc.gpsimd.dma_start(out=d_out[i],
                                        in_=wc_tiles[-1][0:8, 0:2048])

            # first layer: 8 fills, NSPLIT sub-fills of 512-col matmuls
            for f in range(8 if stop_after != "dma" else 0):
                for h in range(NSPLIT):
                    psum = ps.tile([128, SUBW], F32, tag="ps", bufs=PS_BUFS,
                                   name="psF")
                    for s2 in range(4 // NSPLIT):
                        t = 4 * f + (4 // NSPLIT) * h + s2
                        rhs = pat_sb[t // 8][:, (t % 8) * 512:(t % 8) * 512 + 512]
                        nc.tensor.matmul(psum[:, s2 * 512:(s2 + 1) * 512],
                                         w0_sb[:], rhs, start=True, stop=True)
                    emit_scatter_sub(psum, h, first_fill_descs(f), 1)

            # recursion layers (program identical across cores; ly only
            # affects the data packed on the host)
            fact_tiles = []
            stop_layer = 0
            if stop_after == "dma":
                stop_layer = -1
            elif stop_after == "first":
                stop_layer = 1
            elif stop_after and stop_after.startswith("l") and \
                    stop_after != "l5":
                stop_layer = int(stop_after[1]) + 1
            if stop_layer == 1:
                emit_stop(1)
            def emit_fill(l, f):
                M = LAYER_M[l]
                npos = LAYER_NPOS[l]
                slots = layer_slots(l, 0)
                w_slot = min(npos, 512)
                sph = SUBW // w_slot
                psum_hs = []
                for h in range(NSPLIT):
                    psum = ps.tile([128, SUBW], F32, tag="ps",
                                   bufs=PS_BUFS, name=f"psl{l}")
                    psum_hs.append(psum)
                    for sj in range(sph):
                        ib, m, chk = slots[(NSPLIT * f + h) * sph + sj]
                        ci, cb = wchunk_of(l, ib)
                        wt = wc_tiles[ci]
                        colg = ib * npos + chk * 512
                        g, loc = colg // TILE_W, colg % TILE_W
                        mh = m * 128 if M == 256 else 0
                        pslice = psum[:, sj * w_slot:(sj + 1) * w_slot]
                        for p in range(2):
                            lhsT = wt[:, cb + p * M + mh:
                                      cb + p * M + mh + 128]
                            rhs = act_tile(l, p, g)[:, loc:loc + w_slot]
                            nc.tensor.matmul(pslice, lhsT, rhs,
                                             start=(p == 0), stop=(p == 1))
                    if l != 5:
                        emit_scatter_sub(psum, h,
                                         layer_fill_descs(l, f, 0), l + 1)
                if l == 5:
                    ft = sb.tile([128, FILL_W], BF, tag="mid", bufs=5,
                                 name=f"fact{f}")
                    for h in range(NSPLIT):
                        relu_copy(ft[:, h * SUBW:(h + 1) * SUBW],
                                  psum_hs[h][:])
                    fact_tiles.append(ft)
                    if stop_after == "l5":
                        nc.gpsimd.dma_start(out=d_out[f], in_=ft[0:8, :])

            max_l = 5 if not stop_layer else max(1, stop_layer) - 1
            if max_l >= 1:
                for f in range(4):
                    emit_fill(1, f)
                for F in range(7):
                    for l in range(2, max_l + 1):
                        f = F - (l - 2)
                        if 0 <= f <= 3:
                            emit_fill(l, f)
                if 1 < stop_layer < 6:
                    emit_stop(stop_layer)

            # final layer: block-diag pairs, K=128, M=8
            for fi in range(4 if stop_after is None else 0):
                fo = sb.tile([8, FILL_W], F32, tag="fo", bufs=2,
                             name=f"fout{fi}")
                for h in range(NSPLIT):
                    psF = ps.tile([128, SUBW], F32, tag="ps",
                                  bufs=PS_BUFS, name="psfin")
                    for s in range(64 // NSPLIT):
                        idx = 64 * fi + (64 // NSPLIT) * h + s
                        c0 = (idx - 64 * fi) * 32
                        rhs = fact_tiles[fi][:, c0:c0 + 32]
                        lhsT = wf_sb[:, idx * 8:idx * 8 + 8]
                        nc.tensor.matmul(psF[0:8, s * 32:(s + 1) * 32],
                                         lhsT, rhs, start=True, stop=True)
                    nc.vector.tensor_scalar_max(
                        fo[:, h * SUBW:(h + 1) * SUBW], psF[0:8, :], 0.0)
                nc.scalar.dma_start(out=d_out[fi], in_=fo[:])
    nc.finalize()
    return nc


# ------------------------------------------------------------------ kernel()
def _pack_in_maps(inputs):
    pat = pack_patches(inputs["input_data"])
    in_maps = []
    for c in range(N_CORES):
        u0, v0, ly, blocks = core_geom(c)
        m = {"patches": pat,
             "w0": pack_first(inputs["W0"], u0, v0),
             "wf": pack_wf(inputs["Wf"], blocks[5]),
             "wcat": pack_wcat(inputs, blocks, ly)}
        in_maps.append(m)
    return in_maps


def kernel(**inputs):
    exp = {"input_data": (B, C, H, W), "W0": (C, KO, 4, 2, 2), "b0": (C, KO),
           "Wf": (C, 64, 64, 4, NCH), "bf": (C, 64, 64, 4)}
    for l in range(1, 6):
        G = 2 ** l
        exp[f"W{l}"] = (C, G, G, KO, NCH, 2, 2)
        exp[f"b{l}"] = (C, G, G, KO)
    ok = all(tuple(np.shape(inputs.get(k, ()))) == v for k, v in exp.items())
    biases_zero = all(not np.any(np.asarray(inputs[k]))
                      for k in inputs if k.startswith("b"))
    if not ok or not biases_zero:
        return _numpy_reference(inputs)

    from concourse.bass_utils import run_bass_kernel_spmd

    if "nc" not in _NC_CACHE:
        _NC_CACHE["nc"] = build_nc()
    res = run_bass_kernel_spmd(_NC_CACHE["nc"], _pack_in_maps(inputs),
                               core_ids=list(range(N_CORES)))
    return decode_outputs({c: res.results[c]["fout"] for c in range(N_CORES)})
